# revision 17
# baseline (speedup 1.0000x reference)
"""Trainium2 Bass kernel for 2-layer GAT (nn_GAT_34832184770812).

Strategy (8 NeuronCores, dst-node sharded):
- Each core owns 1250 dst nodes; node ids are rotated per core so own nodes
  are local rows 0:1250 (keeps the SPMD program identical across cores).
- Phase A: T1 = features @ [W1 | W1@al1 | W1@ar1] (bf16, replicated) ->
  DRAM gather table t1tab[N, 384] (feat 256 | el 256:260 | er 260:264 |
  pad).  Batched: one featT load + one full-row table write per 512 nodes.
- Phase B (layer-1 edge phase): edges sorted by dst window (128 dst rows per
  window, padded to 128-edge chunks, chunk counts uniform across cores).
  Per 1024-edge superchunk: ONE dma_gather of src rows; er per edge via a
  one-hot indicator matmul (Ind loaded 2 superchunks per DMA); attention
  e = lrelu(el+er), ex = exp(e) (softmax shift-invariance); messages scaled
  by ex (pair-duplicated ex for DVE 2x mode); segment sum over dst via
  indicator-transpose matmul (IndT resident in SBUF, single preload DMA),
  with ex columns producing softmax denominators in the same psum.
- Window finalize: normalize, ELU, transpose (PE), T2own = h @ W2p.
- AllGather T2own (bf16) -> T2all [N, 128] global gather table (Shared).
- Phase D (layer-2 edge phase): same structure, 1 head, 47 feats.
- log_softmax per window, output [1250, 47] f32 per core, host concat.

Perf notes: each dma_start costs ~630ns on the shared HWDGE unit, so loads
are batched aggressively (gidx 4 sc/DMA, ind 2 sc/DMA, er preloaded for all
windows in one DMA per layer).  The big ex*feat multiply uses an explicitly
pair-duplicated ex operand so every DVE operand has innermost stride 1 over
>=2 two-byte elements (2x DVE mode).
"""

import numpy as np
import ml_dtypes

BF16 = ml_dtypes.bfloat16

# problem constants (hardcoded per contract)
N = 10000
E = 320000
IN_FEATS = 256
H = 4
D = 64
HD = 256
OUTF = 47
NEG = 0.2
NCORES = 8
OWN = N // NCORES          # 1250
P = 128
NWIN = (OWN + P - 1) // P  # 10 windows (last has 98 nodes)
WIN_SIZES = [min(P, OWN - P * w) for w in range(NWIN)]
K = 8                      # chunks per superchunk
ROW1 = 384                 # (unused) legacy bf16 row
ROW1F = 512                # T1 gather row (fp8 bytes): feat 0:256 | el bf16 @256:264 | er bf16 @264:272 | pad
ROW2 = 128                 # T2 gather row (bf16): feat 0:47 | el2 47 | er2 48 | ex2 49 | pad
T2OWN_ROWS = NWIN * P      # 1280 (rows 1250:1280 zeroed)
GIDX_B = 4                 # superchunks of gather indices per DMA
IND_B = 2                  # superchunks of Ind per DMA

_CACHE = {}


# ----------------------------------------------------------------------------
# host-side graph preprocessing
# ----------------------------------------------------------------------------

def _prep_graph(src, dst):
    """Per-core edge partition, window sort, uniform padding, one-hots."""
    src = np.asarray(src).astype(np.int64)
    dst = np.asarray(dst).astype(np.int64)
    core_of = dst // OWN
    per_core = []
    for c in range(NCORES):
        sel = np.nonzero(core_of == c)[0]
        dl = dst[sel] - OWN * c                       # local dst in [0, OWN)
        sl = (src[sel] - OWN * c) % N                 # local src
        order = np.argsort(dl, kind="stable")
        per_core.append((dl[order], sl[order], src[sel][order]))

    # uniform chunks per window across cores
    cw = []
    bounds = []
    for c in range(NCORES):
        dl = per_core[c][0]
        b = np.searchsorted(dl, [P * w for w in range(NWIN + 1)])
        bounds.append(b)
    for w in range(NWIN):
        mx = max(bounds[c][w + 1] - bounds[c][w] for c in range(NCORES))
        cw.append((int(mx) + P - 1) // P)
    nchunk = sum(cw)
    nsc = (nchunk + K - 1) // K
    pad_chunks = nsc * K - nchunk
    cw[-1] += pad_chunks
    nchunk = nsc * K

    chunk_win = []
    for w in range(NWIN):
        chunk_win += [w] * cw[w]

    ES = nchunk * P
    out = []
    for c in range(NCORES):
        dl, sl, sg = per_core[c]
        b = bounds[c]
        src_loc = np.zeros(ES, np.int16)
        src_glb = np.zeros(ES, np.int16)
        dstrow = np.full(ES, -1, np.int32)            # -1 = dummy
        pos = 0
        for w in range(NWIN):
            e0, e1 = b[w], b[w + 1]
            n = e1 - e0
            src_loc[pos : pos + n] = sl[e0:e1]
            src_glb[pos : pos + n] = sg[e0:e1]
            dstrow[pos : pos + n] = dl[e0:e1] - P * w
            pos += cw[w] * P
        # one-hot indicators
        ind = np.zeros((ES, P), ml_dtypes.float8_e4m3)  # [chunk*128 + dstrow, e]
        indt = np.zeros((ES, P), BF16)                # [chunk*128 + e, dstrow]
        ch = np.arange(ES) // P
        e_in = np.arange(ES) % P
        real = dstrow >= 0
        r = np.nonzero(real)[0]
        ind[ch[r] * P + dstrow[r], e_in[r]] = 1
        indt[ch[r] * P + e_in[r], dstrow[r]] = 1
        # repack: ind -> [nsc*P, K*P] (row p of sc = concat_j ind[(sc*K+j)*P+p]);
        # indt -> [P, nchunk*P] (row p = concat_ci indt[ci*P+p])
        ind_sc = np.ascontiguousarray(
            ind.reshape(nsc, K, P, P).transpose(0, 2, 1, 3).reshape(nsc * P, K * P))
        indt_r = np.ascontiguousarray(
            indt.reshape(nchunk, P, P).transpose(1, 0, 2).reshape(P, nchunk * P))
        # dma_gather wrapped idx layout per superchunk
        def wrap(ids):
            lay = np.zeros((nsc * P, K * P // 16), np.int16)
            for sc in range(nsc):
                blk = ids[sc * K * P : (sc + 1) * K * P]
                wr = np.zeros((16, K * P // 16), np.int16)
                kk = np.arange(K * P)
                wr[kk % 16, kk // 16] = blk
                lay[sc * P : (sc + 1) * P] = np.tile(wr, (8, 1))
            return lay
        # layer-2 table row in the split-AllGather layout: half A holds local
        # rows 0:640 of every core (concat core-major), half B rows 640:1280
        sg64 = src_glb.astype(np.int64)
        cg = sg64 // OWN
        lg = sg64 % OWN
        sg2 = np.where(lg < 640, cg * 640 + lg, 5 * NCORES * P + cg * 640 + (lg - 640))
        out.append(dict(gidx1=wrap(src_loc), gidx2=wrap(sg2.astype(np.int16)),
                        ind=ind_sc, indt=indt_r))
    return out, cw, nchunk, nsc, chunk_win


# ----------------------------------------------------------------------------
# program build
# ----------------------------------------------------------------------------

def build_program(nchunk, nsc, chunk_win, reps=1):
    import concourse.tile as tile
    from concourse import bacc, mybir
    from concourse.masks import make_identity

    NT = (N + P - 1) // P                              # 79 node tiles
    # chunk boundaries: first/last chunk of each window
    win_first = {}
    win_last = {}
    for ci, w in enumerate(chunk_win):
        if w not in win_first:
            win_first[w] = ci
        win_last[w] = ci

    nc = bacc.Bacc("TRN2", target_bir_lowering=False, debug=False, num_devices=NCORES)
    dt = mybir.dt
    featT = nc.declare_dram_parameter("featT", [IN_FEATS, N], dt.bfloat16, isOutput=False)
    W1p = nc.declare_dram_parameter("W1p", [IN_FEATS, 264], dt.bfloat16, isOutput=False)
    W2p = nc.declare_dram_parameter("W2p", [HD, 52], dt.bfloat16, isOutput=False)
    gidx1 = nc.declare_dram_parameter("gidx1", [nsc * P, K * P // 16], dt.int16, isOutput=False)
    gidx2 = nc.declare_dram_parameter("gidx2", [nsc * P, K * P // 16], dt.int16, isOutput=False)
    indp = nc.declare_dram_parameter("ind", [nsc * P, K * P], dt.float8e4, isOutput=False)
    indtp = nc.declare_dram_parameter("indt", [P, nchunk * P], dt.bfloat16, isOutput=False)
    outp = nc.declare_dram_parameter("out", [OWN, OUTF], dt.float32, isOutput=True)

    t1tab = nc.dram_tensor("t1tab", [N, ROW1F], dt.float8e4)
    t2own = nc.dram_tensor("t2own", [T2OWN_ROWS, ROW2], dt.bfloat16)

    with tile.TileContext(nc) as tc:
        with (
            tc.tile_pool(name="const", bufs=1) as constp,
            tc.tile_pool(name="res", bufs=1) as respool,
            tc.tile_pool(name="dram", bufs=1, space="DRAM") as dramp,
        ):
            ident = constp.tile([P, P], dt.float32)
            make_identity(nc, ident[:])
            zero52 = constp.tile([P, 52], dt.bfloat16)
            nc.vector.memset(zero52[:], 0)

            # resident IndT (shared by both layers) — single preload DMA
            indt_all = respool.tile([P, nchunk * P], dt.bfloat16, tag="res")
            nc.sync.dma_start(out=indt_all[:], in_=indtp[:, :])

            t2all2 = dramp.tile([2 * NCORES * 640, ROW2], dt.bfloat16, tag="t2all2")

            for r in range(reps):
                last = r == reps - 1
                # ---------------- phase A: T1 table ----------------
                with (
                    tc.tile_pool(name="pa", bufs=3) as pa,
                    tc.tile_pool(name="paps", bufs=8, space="PSUM") as paps,
                    tc.tile_pool(name="w1pool", bufs=1) as w1pool,
                ):
                    w1t = w1pool.tile([P, 264], dt.bfloat16, tag="w1a")
                    nc.sync.dma_start(out=w1t[:], in_=W1p[0:P, :])
                    w1b = w1pool.tile([P, 264], dt.bfloat16, tag="w1b")
                    nc.sync.dma_start(out=w1b[:], in_=W1p[P:IN_FEATS, :])
                    # 9 groups of 8 full node tiles + tail (6 full + 16 cols)
                    GA = 8
                    for it in range((NT + GA - 1) // GA):
                        c0 = it * GA * P
                        m = min(GA * P, N - c0)
                        nfull = m // P                 # full 128-col subtiles
                        mtail = m - nfull * P          # leftover cols (16 at the end)
                        lt = pa.tile([P, 2, GA * P], dt.bfloat16, tag="lt")
                        nc.scalar.dma_start(
                            out=lt[:, :, 0:m],
                            in_=featT[:, c0 : c0 + m].rearrange("(h p) m -> p h m", p=P),
                        )
                        row = pa.tile([P, GA, ROW1F], dt.float8e4, tag="row")
                        rowb = row[:].bitcast(dt.bfloat16)
                        for q in range((m + P - 1) // P):
                            mm = min(P, m - q * P)
                            ps = paps.tile([P, 264], dt.float32, space="PSUM", tag="paps")
                            nc.tensor.matmul(ps[0:mm, :], lhsT=lt[:, 0, q * P : q * P + mm],
                                             rhs=w1t[:], start=True, stop=False)
                            nc.tensor.matmul(ps[0:mm, :], lhsT=lt[:, 1, q * P : q * P + mm],
                                             rhs=w1b[:], start=False, stop=True)
                            if q % 2 == 0:
                                nc.vector.tensor_copy(row[0:mm, q, 0:256], ps[0:mm, 0:256])
                            else:
                                nc.scalar.activation(row[0:mm, q, 0:256], ps[0:mm, 0:256],
                                                     mybir.ActivationFunctionType.Copy)
                            nc.vector.tensor_copy(rowb[0:mm, q, 128:136], ps[0:mm, 256:264])
                        if nfull:
                            nc.sync.dma_start(
                                out=t1tab[c0 : c0 + nfull * P, :].rearrange(
                                    "(q p) c -> p q c", p=P),
                                in_=row[:, 0:nfull, :],
                            )
                        if mtail:
                            nc.sync.dma_start(
                                out=t1tab[c0 + nfull * P : c0 + m, :],
                                in_=row[0:mtail, nfull, :],
                            )

                # zero t2own pad rows once
                nc.sync.dma_start(out=t2own[OWN:T2OWN_ROWS, 0:52], in_=zero52[0 : T2OWN_ROWS - OWN, :])

                # ---------------- phase B: layer-1 edge phase ----------------
                with (
                    tc.tile_pool(name="pb", bufs=8) as pb,
                    tc.tile_pool(name="pbi", bufs=2) as pbi,
                    tc.tile_pool(name="pbw", bufs=1) as pbw,
                    tc.tile_pool(name="pbfin", bufs=2) as pbfin,
                    tc.tile_pool(name="wps", bufs=2, space="PSUM") as wps,
                    tc.tile_pool(name="erps", bufs=4, space="PSUM") as erps,
                    tc.tile_pool(name="finps", bufs=2, space="PSUM") as finps,
                    tc.tile_pool(name="w2pool", bufs=1) as w2pool,
                ):
                    w2t = w2pool.tile([P, 52], dt.bfloat16, tag="w2a")
                    nc.sync.dma_start(out=w2t[:], in_=W2p[0:P, :])
                    w2b = w2pool.tile([P, 52], dt.bfloat16, tag="w2b")
                    nc.sync.dma_start(out=w2b[:], in_=W2p[P:HD, :])
                    # er for all windows in one strided DMA (after phase A)
                    er_all = pbw.tile([P, NWIN, 4], dt.bfloat16, tag="erall")
                    nc.scalar.dma_start(
                        out=er_all[:],
                        in_=t1tab[0 : NWIN * P, 264:272].bitcast(dt.bfloat16).rearrange(
                            "(w p) c -> p w c", p=P),
                    )

                    win_psum = None
                    it_t = None
                    ind_t = None
                    for sc in range(nsc):
                        if sc % GIDX_B == 0:
                            nb = min(GIDX_B, nsc - sc)
                            it_t = pb.tile([P, GIDX_B, K * P // 16], dt.int16, tag="idx")
                            nc.scalar.dma_start(
                                out=it_t[:, 0:nb, :],
                                in_=gidx1[sc * P : (sc + nb) * P, :].rearrange(
                                    "(s p) c -> p s c", p=P),
                            )
                        if sc % IND_B == 0:
                            nb = min(IND_B, nsc - sc)
                            ind_t = pbi.tile([P, IND_B, K, P], dt.float8e4, tag="ind")
                            nc.sync.dma_start(
                                out=ind_t[:, 0:nb, :, :],
                                in_=indp[sc * P : (sc + nb) * P, :].rearrange(
                                    "(s p) (j q) -> p s j q", p=P, q=P),
                            )
                        g = pb.tile([P, K, ROW1F], dt.float8e4, tag="g", bufs=6)
                        nc.gpsimd.dma_gather(g[:], t1tab[:, :], it_t[:, sc % GIDX_B, :],
                                             K * P, K * P, ROW1F)
                        gb = g[:].bitcast(dt.bfloat16)
                        er_psum = erps.tile([P, K * 4], dt.float32, space="PSUM", tag="erp")
                        for j in range(K):
                            ci = sc * K + j
                            w = chunk_win[ci]
                            nc.tensor.matmul(
                                er_psum[:, j * 4 : (j + 1) * 4],
                                lhsT=ind_t[:, sc % IND_B, j, :], rhs=er_all[:, w, :],
                                start=True, stop=True,
                            )
                        att = pb.tile([P, K, 4], dt.float32, tag="att")
                        nc.vector.tensor_tensor(
                            out=att[:], in0=gb[:, :, 128:132],
                            in1=er_psum[:].rearrange("p (c h) -> p c h", c=K),
                            op=mybir.AluOpType.add,
                        )
                        att2 = pb.tile([P, K, 4], dt.float32, tag="att2")
                        nc.vector.tensor_scalar_mul(att2[:], att[:], NEG)
                        nc.vector.tensor_tensor(out=att[:], in0=att[:], in1=att2[:], op=mybir.AluOpType.max)
                        # fp8 feat -> bf16 on the (idle) Act engine; Copy is in
                        # every act table set so this never forces a table switch
                        gfb = pb.tile([P, K, HD], dt.bfloat16, tag="gfb", bufs=6)
                        nc.scalar.activation(gfb[:], g[:, :, 0:HD],
                                             mybir.ActivationFunctionType.Copy)
                        # ex pair-duplicated (separate contiguous tile so the AP
                        # optimizer can merge (c,h) and keep the mult 4D/2x)
                        ex2 = pb.tile([P, K, 4, 2], dt.bfloat16, tag="ex2")
                        nc.scalar.activation(ex2[:, :, :, 0:1], att[:, :, :, None],
                                             mybir.ActivationFunctionType.Exp)
                        nc.vector.tensor_copy(ex2[:, :, :, 1:2], ex2[:, :, :, 0:1])
                        # messages (bf16) with ex in cols 256:260 (softmax denominators)
                        msg = pb.tile([P, K, 260], dt.bfloat16, tag="msg", bufs=6)
                        nc.vector.tensor_copy(msg[:, :, 256:260], ex2[:, :, :, 0])
                        nc.vector.tensor_tensor(
                            out=msg[:, :, 0:HD].rearrange("p c (h d two) -> p c h d two", h=H, two=2),
                            in0=gfb[:].rearrange("p c (h d two) -> p c h d two", h=H, two=2),
                            in1=ex2[:, :, :, None, :].broadcast_to([P, K, 4, D // 2, 2]),
                            op=mybir.AluOpType.mult,
                        )
                        for j in range(K):
                            ci = sc * K + j
                            w = chunk_win[ci]
                            if ci == win_first[w]:
                                win_psum = wps.tile([P, 260], dt.float32, space="PSUM", tag="acc")
                            nc.tensor.matmul(
                                win_psum[:],
                                lhsT=indt_all[:, ci * P : (ci + 1) * P],
                                rhs=msg[:, j, :],
                                start=(ci == win_first[w]),
                                stop=(ci == win_last[w]),
                            )
                            if ci == win_last[w]:
                                m = WIN_SIZES[w]
                                # normalize: h = msg / max(denom, eps)
                                den = pbfin.tile([P, 4], dt.float32, tag="den")
                                nc.vector.tensor_scalar_max(den[:], win_psum[:, 256:260], 1e-9)
                                rec = pbfin.tile([P, 4], dt.float32, tag="rec")
                                nc.vector.reciprocal(rec[:], den[:])
                                h_sb = pbfin.tile([P, HD], dt.float32, tag="hsb")
                                nc.vector.tensor_tensor(
                                    out=h_sb[:].rearrange("p (h d) -> p h d", h=H),
                                    in0=win_psum[:, 0:HD].rearrange("p (h d) -> p h d", h=H),
                                    in1=rec[:, :, None].broadcast_to([P, H, D]),
                                    op=mybir.AluOpType.mult,
                                )
                                # ELU: relu(h) + exp(min(h,0)) - 1
                                hneg = pbfin.tile([P, HD], dt.float32, tag="hneg")
                                nc.vector.tensor_scalar_min(hneg[:], h_sb[:], 0.0)
                                hexp = pbfin.tile([P, HD], dt.float32, tag="hexp")
                                nc.scalar.activation(hexp[:], hneg[:], mybir.ActivationFunctionType.Exp)
                                nc.vector.tensor_scalar_max(h_sb[:], h_sb[:], 0.0)
                                nc.vector.tensor_tensor(out=h_sb[:], in0=h_sb[:], in1=hexp[:], op=mybir.AluOpType.add)
                                nc.vector.tensor_scalar_add(h_sb[:], h_sb[:], -1.0)
                                # transpose h (2x PE) -> hT bf16
                                hT = pbfin.tile([P, 2, P], dt.bfloat16, tag="hT")
                                for half in range(2):
                                    tp = finps.tile([P, P], dt.float32, space="PSUM", tag="fin")
                                    nc.tensor.transpose(out=tp[:, 0:m], in_=h_sb[0:m, half * P : (half + 1) * P], identity=ident[0:m, 0:m])
                                    nc.vector.tensor_copy(hT[:, half, 0:m], tp[:, 0:m])
                                # T2own rows = h @ W2p
                                t2ps = finps.tile([P, 52], dt.float32, space="PSUM", tag="fin")
                                nc.tensor.matmul(t2ps[0:m, :], lhsT=hT[:, 0, 0:m], rhs=w2t[:], start=True, stop=False)
                                nc.tensor.matmul(t2ps[0:m, :], lhsT=hT[:, 1, 0:m], rhs=w2b[:], start=False, stop=True)
                                t2row = pbfin.tile([P, 52], dt.bfloat16, tag="t2row")
                                nc.vector.tensor_copy(t2row[0:m, :], t2ps[0:m, :])
                                nc.sync.dma_start(out=t2own[w * P : w * P + m, 0:52], in_=t2row[0:m, :])
                                if w == 4:
                                    # first-half allgather overlaps windows 5-9
                                    t2b_a = dramp.tile([640, ROW2], dt.bfloat16, tag="t2ba")
                                    nc.gpsimd.dma_start(out=t2b_a[:], in_=t2own[0:640, :])
                                    nc.gpsimd.collective_compute(
                                        "AllGather",
                                        mybir.AluOpType.bypass,
                                        replica_groups=[list(range(NCORES))],
                                        ins=[t2b_a.opt()],
                                        outs=[t2all2[0 : NCORES * 640, :].opt()],
                                    )

                # ---------------- phase C: allgather T2 (second half) ----------------
                t2b_b = dramp.tile([640, ROW2], dt.bfloat16, tag="t2bb")
                nc.gpsimd.dma_start(out=t2b_b[:], in_=t2own[640:T2OWN_ROWS, :])
                nc.gpsimd.collective_compute(
                    "AllGather",
                    mybir.AluOpType.bypass,
                    replica_groups=[list(range(NCORES))],
                    ins=[t2b_b.opt()],
                    outs=[t2all2[NCORES * 640 : 2 * NCORES * 640, :].opt()],
                )

                # ---------------- phase D: layer-2 edge phase ----------------
                with (
                    tc.tile_pool(name="pd", bufs=8) as pd,
                    tc.tile_pool(name="pdi", bufs=2) as pdi,
                    tc.tile_pool(name="pdw", bufs=1) as pdw,
                    tc.tile_pool(name="pdfin", bufs=2) as pdfin,
                    tc.tile_pool(name="wps2", bufs=2, space="PSUM") as wps2,
                    tc.tile_pool(name="erps2", bufs=4, space="PSUM") as erps2,
                ):
                    logit_all = pdw.tile([P, NWIN, OUTF], dt.float32, tag="logall")
                    se_all = pdw.tile([P, NWIN], dt.float32, tag="seall")
                    er2_all = pdw.tile([P, NWIN, 1], dt.bfloat16, tag="er2all")
                    nc.scalar.dma_start(
                        out=er2_all[:],
                        in_=t2own[0 : NWIN * P, 48:49].rearrange("(w p) c -> p w c", p=P),
                    )
                    win_psum2 = None
                    it_t = None
                    ind_t = None
                    for sc in range(nsc):
                        if sc % GIDX_B == 0:
                            nb = min(GIDX_B, nsc - sc)
                            it_t = pd.tile([P, GIDX_B, K * P // 16], dt.int16, tag="idx2")
                            nc.scalar.dma_start(
                                out=it_t[:, 0:nb, :],
                                in_=gidx2[sc * P : (sc + nb) * P, :].rearrange(
                                    "(s p) c -> p s c", p=P),
                            )
                        if sc % IND_B == 0:
                            nb = min(IND_B, nsc - sc)
                            ind_t = pdi.tile([P, IND_B, K, P], dt.float8e4, tag="ind2")
                            nc.sync.dma_start(
                                out=ind_t[:, 0:nb, :, :],
                                in_=indp[sc * P : (sc + nb) * P, :].rearrange(
                                    "(s p) (j q) -> p s j q", p=P, q=P),
                            )
                        g2 = pd.tile([P, K, ROW2], dt.bfloat16, tag="g2")
                        nc.gpsimd.dma_gather(g2[:], t2all2[:, :], it_t[:, sc % GIDX_B, :],
                                             K * P, K * P, ROW2)
                        er_psum2 = erps2.tile([P, K], dt.float32, space="PSUM", tag="erp2")
                        for j in range(K):
                            ci = sc * K + j
                            w = chunk_win[ci]
                            nc.tensor.matmul(
                                er_psum2[:, j : j + 1],
                                lhsT=ind_t[:, sc % IND_B, j, :], rhs=er2_all[:, w, :],
                                start=True, stop=True,
                            )
                        att = pd.tile([P, K], dt.float32, tag="attl2")
                        nc.vector.tensor_tensor(
                            out=att[:, :, None], in0=g2[:, :, 47:48], in1=er_psum2[:, :, None],
                            op=mybir.AluOpType.add,
                        )
                        att2 = pd.tile([P, K], dt.float32, tag="attl2b")
                        nc.vector.tensor_scalar_mul(att2[:], att[:], NEG)
                        nc.vector.tensor_tensor(out=att[:], in0=att[:], in1=att2[:], op=mybir.AluOpType.max)
                        ex2 = pd.tile([P, K, 2], dt.bfloat16, tag="ex2d")
                        nc.scalar.activation(ex2[:, :, 0:1], att[:, :, None],
                                             mybir.ActivationFunctionType.Exp)
                        nc.vector.tensor_copy(ex2[:, :, 1:2], ex2[:, :, 0:1])
                        nc.vector.tensor_copy(g2[:, :, 49:50], ex2[:, :, 0:1])
                        nc.vector.tensor_tensor(
                            out=g2[:, :, 0:48].rearrange("p c (d two) -> p c d two", two=2),
                            in0=g2[:, :, 0:48].rearrange("p c (d two) -> p c d two", two=2),
                            in1=ex2[:, :, None, :].broadcast_to([P, K, 24, 2]),
                            op=mybir.AluOpType.mult,
                        )
                        for j in range(K):
                            ci = sc * K + j
                            w = chunk_win[ci]
                            if ci == win_first[w]:
                                win_psum2 = wps2.tile([P, 50], dt.float32, space="PSUM", tag="acc2")
                            nc.tensor.matmul(
                                win_psum2[:],
                                lhsT=indt_all[:, ci * P : (ci + 1) * P],
                                rhs=g2[:, j, 0:50],
                                start=(ci == win_first[w]),
                                stop=(ci == win_last[w]),
                            )
                            if ci == win_last[w]:
                                den = pdfin.tile([P, 1], dt.float32, tag="den2")
                                nc.vector.tensor_scalar_max(den[:], win_psum2[:, 49:50], 1e-9)
                                rec = pdfin.tile([P, 1], dt.float32, tag="rec2")
                                nc.vector.reciprocal(rec[:], den[:])
                                nc.vector.tensor_scalar(
                                    out=logit_all[:, w, :], in0=win_psum2[:, 0:OUTF],
                                    scalar1=rec[:, 0:1], scalar2=None,
                                    op0=mybir.AluOpType.mult,
                                )
                                mx = pdfin.tile([P, 1], dt.float32, tag="mx")
                                nc.vector.tensor_reduce(mx[:], logit_all[:, w, :], mybir.AxisListType.X, mybir.AluOpType.max)
                                nc.vector.tensor_scalar(
                                    out=logit_all[:, w, :], in0=logit_all[:, w, :],
                                    scalar1=mx[:, 0:1], scalar2=None,
                                    op0=mybir.AluOpType.subtract,
                                )
                                exps = pdfin.tile([P, OUTF], dt.float32, tag="exps")
                                nc.scalar.activation(exps[:], logit_all[:, w, :],
                                                     mybir.ActivationFunctionType.Exp,
                                                     accum_out=se_all[:, w : w + 1])
                    # batched log-sum-exp epilogue (one Ln table switch per rep)
                    lse_all = pdw.tile([P, NWIN], dt.float32, tag="lseall")
                    nc.scalar.activation(lse_all[:], se_all[:], mybir.ActivationFunctionType.Ln)
                    if last:
                        for w in range(NWIN):
                            m = WIN_SIZES[w]
                            logit = pdfin.tile([P, OUTF], dt.float32, tag="logit")
                            nc.vector.tensor_scalar(
                                out=logit[:], in0=logit_all[:, w, :],
                                scalar1=lse_all[:, w : w + 1], scalar2=None,
                                op0=mybir.AluOpType.subtract,
                            )
                            nc.sync.dma_start(out=outp[w * P : w * P + m, :], in_=logit[0:m, :])
    nc.compile()
    return nc


# ----------------------------------------------------------------------------
# host entry
# ----------------------------------------------------------------------------

def _host_inputs(features, src, dst, W1, al1, ar1, W2, al2, ar2):
    feats = np.asarray(features, np.float32)
    W1 = np.asarray(W1, np.float32)
    W2 = np.asarray(W2, np.float32)
    al1 = np.asarray(al1, np.float32)
    ar1 = np.asarray(ar1, np.float32)
    al2 = np.asarray(al2, np.float32)
    ar2 = np.asarray(ar2, np.float32)

    Wl1 = np.stack([W1[:, h * D : (h + 1) * D] @ al1[h] for h in range(H)], axis=1)
    Wr1 = np.stack([W1[:, h * D : (h + 1) * D] @ ar1[h] for h in range(H)], axis=1)
    W1p = np.concatenate([W1, Wl1, Wr1], axis=1).astype(BF16)          # [256, 264]
    Wl2 = (W2 @ al2[0])[:, None]
    Wr2 = (W2 @ ar2[0])[:, None]
    W2p = np.concatenate([W2, Wl2, Wr2, np.zeros((HD, 3), np.float32)], axis=1).astype(BF16)  # [256, 52]

    graph, cw, nchunk, nsc, chunk_win = _prep_graph(src, dst)
    featT = np.ascontiguousarray(feats.T)                               # [256, N]
    in_maps = []
    for c in range(NCORES):
        featTl = np.roll(featT, -OWN * c, axis=1)                       # local node order
        in_maps.append(dict(
            featT=featTl.astype(BF16),
            W1p=W1p, W2p=W2p,
            gidx1=graph[c]["gidx1"], gidx2=graph[c]["gidx2"],
            ind=graph[c]["ind"], indt=graph[c]["indt"],
        ))
    return in_maps, nchunk, nsc, chunk_win


def kernel(features, src, dst, W1, al1, ar1, W2, al2, ar2):
    from concourse.bass_utils import run_bass_kernel_spmd

    in_maps, nchunk, nsc, chunk_win = _host_inputs(
        features, src, dst, W1, al1, ar1, W2, al2, ar2)
    key = (nchunk, nsc, tuple(chunk_win))
    if key not in _CACHE:
        _CACHE[key] = build_program(nchunk, nsc, chunk_win, reps=1)
    nc = _CACHE[key]
    res = run_bass_kernel_spmd(nc, in_maps, core_ids=list(range(NCORES)))
    return np.concatenate([res.results[c]["out"] for c in range(NCORES)], axis=0)
